# revision 2
# baseline (speedup 1.0000x reference)
"""Column-wise RMS normalization on 8 Trainium2 NeuronCores.

Computes y = x * rsqrt(sum(x*x, axis=0) + eps) for x [32768, 2048] f32.

Sharding: column-parallel — each core owns a contiguous block of 256
columns, making the per-column sum-of-squares entirely core-local (no
collectives). Within a core the shard is viewed as [128 p, 256 t, 256 c]
(row = p*256 + t) so every DMA moves multi-KB contiguous runs per
partition.

fp16 wire format: the rel-err budget (2e-2) dwarfs fp16 rounding
(~3e-4), so the host casts x to fp16 before upload and the device
returns fp16 y — halving HBM traffic vs f32 to 16MB in + 16MB out per
core, the bandwidth floor for this regime. On-device: plain HWDGE DMA
into a persistent fp16 SBUF cache (16MB/core), squares on DVE, column
reduction via TensorE ones-matmuls into PSUM, rsqrt scale broadcast by
a K=1 matmul, then DVE scale-mul straight to fp16 out-tiles.
"""

import numpy as np

import concourse.bacc as bacc
import concourse.bass as bass
import concourse.tile as tile
from concourse import mybir
from concourse.bass_utils import run_bass_kernel_spmd

N, D = 32768, 2048
EPS = 1e-6
NCORES = 8
C = D // NCORES  # 256 columns per core
P = 128          # partitions
T = N // P       # 256 rows per partition
G = 8            # row-group (t) per out-DMA / compute chunk
NG = T // G      # 32 groups

_NC = None


def _build() -> bass.Bass:
    nc = bacc.Bacc("TRN2", target_bir_lowering=False, enable_partition_id=False)
    x = nc.dram_tensor("x", [N, C], mybir.dt.float16, kind="ExternalInput")
    y = nc.dram_tensor("y", [N, C], mybir.dt.float16, kind="ExternalOutput")
    xv = x[:, :].rearrange("(p t) c -> p t c", p=P)
    yv = y[:, :].rearrange("(p t) c -> p t c", p=P)

    with tile.TileContext(nc) as tc:
        with (
            tc.tile_pool(name="cache", bufs=1) as cachep,
            tc.tile_pool(name="consts", bufs=1) as consts,
            tc.tile_pool(name="sq", bufs=2) as sqp,
            tc.tile_pool(name="outs", bufs=4) as outp,
            tc.tile_pool(name="scale", bufs=1) as scalep,
            tc.tile_pool(name="ps", bufs=1, space="PSUM") as psp,
        ):
            xc = cachep.tile([P, T, C], mybir.dt.float16)
            ones_col = consts.tile([P, 1], mybir.dt.float16)
            nc.vector.memset(ones_col, 1.0)
            ones_row = consts.tile([1, P], mybir.dt.float32)
            nc.vector.memset(ones_row, 1.0)
            eps_t = consts.tile([P, 1], mybir.dt.float32)
            nc.vector.memset(eps_t, EPS)

            # u_ps holds 2 interleaved partial column-sum vectors (even/odd t)
            u_ps = psp.tile([1, 2 * C], mybir.dt.float32)
            s_ps = psp.tile([P, 1, C], mybir.dt.float32)

            # Pass A: plain HWDGE DMA into the persistent fp16 cache (2MB
            # transfers, 16KB contiguous per partition), square on DVE,
            # reduce over partitions (PE ones-matmul accumulate into PSUM).
            # Ramp the tail down (16, 8, 4, 2, 2) so the final
            # square->matmul chain into the scale computation is short.
            GI = 32
            in_groups = (
                [(j * GI, GI) for j in range(T // GI - 1)]
                + [(T - GI, 16), (T - 16, 8), (T - 8, 4), (T - 4, 2), (T - 2, 2)]
            )
            nmm = T // 2
            k = 0
            for t0, g in in_groups:
                ts_ = slice(t0, t0 + g)
                nc.scalar.dma_start(out=xc[:, ts_, :], in_=xv[:, ts_, :])
                # Tail (g==2) squares get their own slot set so they don't
                # stall on PE consuming the big groups' sq slots.
                if g > 2:
                    sq = sqp.tile([P, g, C], mybir.dt.float16, tag="sq", bufs=2)
                else:
                    sq = sqp.tile([P, g, C], mybir.dt.float16, tag="sqt", bufs=4)
                nc.vector.tensor_mul(sq, xc[:, ts_, :], xc[:, ts_, :])
                for h in range(g // 2):
                    rhs = sq[:, 2 * h : 2 * h + 2, :].rearrange("p t c -> p (t c)")
                    nc.tensor.matmul(
                        u_ps[:, :],
                        lhsT=ones_col[:, :],
                        rhs=rhs,
                        start=(k == 0),
                        stop=(k == nmm - 1),
                    )
                    k += 1

            # Scale: u = even+odd partials; s = 1/sqrt(u+eps) computed on the
            # narrow [1, C] vector, THEN broadcast to all partitions with a
            # K=1 matmul into PSUM. Pass-B muls read s straight from PSUM,
            # which keeps the post-broadcast hop off the critical path.
            u_sb = scalep.tile([1, C], mybir.dt.float32)
            upair = u_ps[:, :].rearrange("p (t c) -> p c t", t=2)
            nc.vector.reduce_sum(u_sb, upair, axis=mybir.AxisListType.X)
            tsq = scalep.tile([1, C], mybir.dt.float32)
            nc.scalar.activation(
                out=tsq[:, :],
                in_=u_sb[:, :],
                func=mybir.ActivationFunctionType.Sqrt,
                bias=eps_t[0:1, :],
                scale=1.0,
            )
            s1 = scalep.tile([1, C], mybir.dt.float32)
            nc.vector.reciprocal_approx_fast(out=s1[:, :], in_=tsq[:, :])
            nc.tensor.matmul(
                s_ps[:, 0, :], lhsT=ones_row[:, :], rhs=s1[:, :], start=True, stop=True
            )

            # Pass B: scale cached x, write fp16 out. Ramp the group size
            # (2,2,2,2,4,4, then 8s) so the first out-DMA launches right
            # after the scale is ready and the DMA queue never starves
            # while the first full-size mul runs.
            out_groups = (
                [(2 * h, 2) for h in range(4)]
                + [(8, 4), (12, 4)]
                + [(2 * G + j * G, G) for j in range(NG - 2)]
            )
            for t0, g in out_groups:
                ts_ = slice(t0, t0 + g)
                ot = outp.tile([P, g, C], mybir.dt.float16, tag="ot")
                nc.vector.tensor_mul(
                    ot, xc[:, ts_, :], s_ps[:, :, :].to_broadcast((P, g, C))
                )
                nc.sync.dma_start(out=yv[:, ts_, :], in_=ot)
    nc.compile()
    return nc


def _get_nc() -> bass.Bass:
    global _NC
    if _NC is None:
        _NC = _build()
    return _NC


def kernel(x) -> np.ndarray:
    x = np.asarray(x, dtype=np.float32)
    assert x.shape == (N, D), x.shape
    xh = x.astype(np.float16)
    nc = _get_nc()
    in_maps = [
        {"x": np.ascontiguousarray(xh[:, i * C : (i + 1) * C])} for i in range(NCORES)
    ]
    try:
        res = run_bass_kernel_spmd(nc, in_maps, core_ids=list(range(NCORES)))
    except Exception:
        # Transient NRT/device hiccups (e.g. a previous process's profiling
        # session left a core wedged) recover after a short pause.
        import time

        time.sleep(5)
        res = run_bass_kernel_spmd(nc, in_maps, core_ids=list(range(NCORES)))
    return np.concatenate(
        [r["y"].astype(np.float32) for r in res.results], axis=1
    )


# revision 3
# speedup vs baseline: 1.1254x; 1.1254x over previous
"""Column-wise RMS normalization on 8 Trainium2 NeuronCores.

Computes y = x * rsqrt(sum(x*x, axis=0) + eps) for x [32768, 2048] f32.

Sharding: column-parallel — each core owns a contiguous block of 256
columns, making the per-column sum-of-squares entirely core-local (no
collectives). Within a core the shard is viewed as [128 p, 256 t, 256 c]
(row = p*256 + t) so every DMA moves multi-KB contiguous runs per
partition.

fp16 wire format: the rel-err budget (2e-2) dwarfs fp16 rounding
(~3e-4), so the host casts x to fp16 before upload and the device
returns fp16 y — halving HBM traffic vs f32 to 16MB in + 16MB out per
core, the bandwidth floor for this regime. On-device: plain HWDGE DMA
into a persistent fp16 SBUF cache (16MB/core), squares on DVE, column
reduction via TensorE ones-matmuls into PSUM, rsqrt scale broadcast by
a K=1 matmul. The scale is then materialized once as a full [P, 16, C]
fp16 SBUF tile so every pass-B mul is an equal-shape stride-1 fp16
tensor_tensor op (broadcast-from-PSUM operands run DVE at half rate).
"""

import numpy as np

import concourse.bacc as bacc
import concourse.bass as bass
import concourse.tile as tile
from concourse import mybir
from concourse.bass_utils import run_bass_kernel_spmd

N, D = 32768, 2048
EPS = 1e-6
NCORES = 8
C = D // NCORES  # 256 columns per core
P = 128          # partitions
T = N // P       # 256 rows per partition
G = 16           # row-group (t) per out-DMA / compute chunk
NG = T // G      # 16 groups

_NC = None


def _build() -> bass.Bass:
    nc = bacc.Bacc("TRN2", target_bir_lowering=False, enable_partition_id=False)
    x = nc.dram_tensor("x", [N, C], mybir.dt.float16, kind="ExternalInput")
    y = nc.dram_tensor("y", [N, C], mybir.dt.float16, kind="ExternalOutput")
    xv = x[:, :].rearrange("(p t) c -> p t c", p=P)
    yv = y[:, :].rearrange("(p t) c -> p t c", p=P)

    with tile.TileContext(nc) as tc:
        with (
            tc.tile_pool(name="cache", bufs=1) as cachep,
            tc.tile_pool(name="consts", bufs=1) as consts,
            tc.tile_pool(name="sq", bufs=2) as sqp,
            tc.tile_pool(name="outs", bufs=4) as outp,
            tc.tile_pool(name="scale", bufs=1) as scalep,
            tc.tile_pool(name="ps", bufs=1, space="PSUM") as psp,
        ):
            xc = cachep.tile([P, T, C], mybir.dt.float16)
            ones_col = consts.tile([P, 1], mybir.dt.float16)
            nc.vector.memset(ones_col, 1.0)
            ones_row = consts.tile([1, P], mybir.dt.float32)
            nc.vector.memset(ones_row, 1.0)
            eps_t = consts.tile([P, 1], mybir.dt.float32)
            nc.vector.memset(eps_t, EPS)

            # u_ps holds 2 interleaved partial column-sum vectors (even/odd t)
            u_ps = psp.tile([1, 2 * C], mybir.dt.float32)
            s_ps = psp.tile([P, 1, C], mybir.dt.float32)

            # Pass A: plain HWDGE DMA into the persistent fp16 cache (1MB
            # transfers, 8KB contiguous per partition), square on DVE,
            # reduce over partitions (PE ones-matmul accumulate into PSUM).
            # Ramp the tail down (8, 4, 2, 2) so the final square->matmul
            # chain into the scale computation is short.
            GI = 16
            in_groups = (
                [(j * GI, GI) for j in range(T // GI - 1)]
                + [(T - GI, 8), (T - 8, 4), (T - 4, 2), (T - 2, 2)]
            )
            nmm = T // 2
            k = 0
            for t0, g in in_groups:
                ts_ = slice(t0, t0 + g)
                nc.scalar.dma_start(out=xc[:, ts_, :], in_=xv[:, ts_, :])
                # Tail (g==2) squares get their own slot set so they don't
                # stall on PE consuming the big groups' sq slots.
                if g > 2:
                    sq = sqp.tile([P, g, C], mybir.dt.float16, tag="sq", bufs=2)
                else:
                    sq = sqp.tile([P, g, C], mybir.dt.float16, tag="sqt", bufs=4)
                nc.vector.tensor_mul(sq, xc[:, ts_, :], xc[:, ts_, :])
                for h in range(g // 2):
                    rhs = sq[:, 2 * h : 2 * h + 2, :].rearrange("p t c -> p (t c)")
                    nc.tensor.matmul(
                        u_ps[:, :],
                        lhsT=ones_col[:, :],
                        rhs=rhs,
                        start=(k == 0),
                        stop=(k == nmm - 1),
                    )
                    k += 1

            # Scale: u = even+odd partials; s = 1/sqrt(u+eps) computed on the
            # narrow [1, C] vector, broadcast to all partitions with a K=1
            # matmul into PSUM, then materialized as a full [P, G, C] fp16
            # SBUF tile so pass-B muls are equal-shape stride-1 fp16 ops.
            u_sb = scalep.tile([1, C], mybir.dt.float32)
            upair = u_ps[:, :].rearrange("p (t c) -> p c t", t=2)
            nc.vector.reduce_sum(u_sb, upair, axis=mybir.AxisListType.X)
            tsq = scalep.tile([1, C], mybir.dt.float32)
            nc.scalar.activation(
                out=tsq[:, :],
                in_=u_sb[:, :],
                func=mybir.ActivationFunctionType.Sqrt,
                bias=eps_t[0:1, :],
                scale=1.0,
            )
            s1 = scalep.tile([1, C], mybir.dt.float32)
            nc.vector.reciprocal_approx_fast(out=s1[:, :], in_=tsq[:, :])
            nc.tensor.matmul(
                s_ps[:, 0, :], lhsT=ones_row[:, :], rhs=s1[:, :], start=True, stop=True
            )
            smax = scalep.tile([P, G, C], mybir.dt.float16)
            nc.vector.tensor_copy(smax, s_ps[:, :, :].to_broadcast((P, G, C)))

            # Pass B: scale cached x, write fp16 out. Ramp the group size
            # (2,2,2,2,4,4,8,8, then 16s) so the first out-DMA launches
            # right after the scale is ready and the DMA queue never starves
            # while the first full-size mul runs.
            out_groups = (
                [(2 * h, 2) for h in range(4)]
                + [(8, 4), (12, 4), (16, 8), (24, 8)]
                + [(2 * G + j * G, G) for j in range(NG - 2)]
            )
            for t0, g in out_groups:
                ts_ = slice(t0, t0 + g)
                ot = outp.tile([P, g, C], mybir.dt.float16, tag="ot")
                nc.vector.tensor_mul(ot, xc[:, ts_, :], smax[:, :g, :])
                nc.sync.dma_start(out=yv[:, ts_, :], in_=ot)
    nc.compile()
    return nc


def _get_nc() -> bass.Bass:
    global _NC
    if _NC is None:
        _NC = _build()
    return _NC


def kernel(x) -> np.ndarray:
    x = np.asarray(x, dtype=np.float32)
    assert x.shape == (N, D), x.shape
    xh = x.astype(np.float16)
    nc = _get_nc()
    in_maps = [
        {"x": np.ascontiguousarray(xh[:, i * C : (i + 1) * C])} for i in range(NCORES)
    ]
    try:
        res = run_bass_kernel_spmd(nc, in_maps, core_ids=list(range(NCORES)))
    except Exception:
        # Transient NRT/device hiccups (e.g. a previous process's profiling
        # session left a core wedged) recover after a short pause.
        import time

        time.sleep(5)
        res = run_bass_kernel_spmd(nc, in_maps, core_ids=list(range(NCORES)))
    return np.concatenate(
        [r["y"].astype(np.float32) for r in res.results], axis=1
    )
